# revision 3
# baseline (speedup 1.0000x reference)
"""Multi-head causal self-attention (B=4, S=2048, E=1024, H=16) on 8 TRN2 cores.

Sharding: hybrid batch x head-group. Core c handles batch b = c//2 and head
group g = c%2 (8 heads each). Each core computes q/k/v projections for its
512 columns of Wq/Wk/Wv, causal flash-style attention for its 8 heads, and a
partial out-projection with its 512 rows of Wo. Host sums the two partial
outputs per batch (the all-reduce of the tensor-parallel split) and
transposes back to [S, E].

All matmuls run in float32r (TF32-like, 1 cycle/row on the PE). Scores are
computed transposed ([k, q] layout) so softmax normalization needs no
attention-matrix transpose: exp() runs on ACT with the padding bias folded
in, the causal mask is a zero-fill affine_select on the exp output, and the
softmax denominator comes from an extra ones-column appended to V.
"""

import numpy as np

import concourse.bass as bass
import concourse.mybir as mybir
import concourse.tile as tile
from concourse import bacc
from concourse.bass_utils import run_bass_kernel_spmd
from concourse.masks import make_identity

f32 = mybir.dt.float32
f32r = mybir.dt.float32r
AF = mybir.ActivationFunctionType
ALU = mybir.AluOpType

B, S, E, H = 4, 2048, 1024, 16
D = E // H          # 64
HL = H // 2         # 8 heads per core
GC = HL * D         # 512 columns per head group
NES = E // 128      # 8 E-slabs
NST = S // 512      # 4 s-tiles of 512
NSS = S // 128      # 16 s-subtiles of 128
NM = GC // 128      # 4 column groups (2 heads each)
NQT = S // 512      # 4 q-tiles per head
NKS = S // 128      # 16 k-subtiles
SCALE = 0.125       # 1/sqrt(D)
NEG = np.float32(-1e30)

_CACHED_NC = None


def _build_bass():
    nc = bacc.Bacc()
    x_d = nc.dram_tensor("x", [S, E], f32, kind="ExternalInput")
    wq_d = nc.dram_tensor("wq", [E, GC], f32r, kind="ExternalInput")
    wk_d = nc.dram_tensor("wk", [E, GC], f32r, kind="ExternalInput")
    wv_d = nc.dram_tensor("wv", [E, GC], f32r, kind="ExternalInput")
    wo_d = nc.dram_tensor("wo", [GC, E], f32r, kind="ExternalInput")
    pad_d = nc.dram_tensor("pad", [128, NKS], f32, kind="ExternalInput")
    aux_d = nc.dram_tensor("aux", [128, 64], f32r, kind="ExternalInput")
    out_d = nc.dram_tensor("outT", [E, S], f32, kind="ExternalOutput")

    with tile.TileContext(nc) as tc:
        with tc.tile_pool(name="consts", bufs=1) as consts, tc.tile_pool(
            name="persist", bufs=1
        ) as persist:
            ident = consts.tile([128, 128], f32, tag="ident")
            make_identity(nc, ident[:])
            pad_sb = consts.tile([128, NKS], f32, tag="pad")
            nc.sync.dma_start(pad_sb[:], pad_d[:])
            aux_sb = consts.tile([128, 64], f32r, tag="aux")
            nc.sync.dma_start(aux_sb[:], aux_d[:])

            qT = persist.tile([128, NM, S], f32r, tag="qT")
            kT = persist.tile([128, NM, S], f32r, tag="kT")
            vsb = persist.tile([128, NSS, HL, D + 1], f32r, tag="v")
            ctxT = persist.tile([128, NM, S], f32r, tag="ctxT")

            # ---- Phase 1: x^T and q/k/v projections -------------------
            with (
                tc.tile_pool(name="wpool", bufs=1) as wpool,
                tc.tile_pool(name="xrow", bufs=2) as xrow,
                tc.tile_pool(name="xtp", bufs=1) as xtp,
                tc.tile_pool(name="xps", bufs=2, space="PSUM") as xps,
                tc.tile_pool(name="pps", bufs=2, space="PSUM") as pps,
            ):
                wq_sb = wpool.tile([128, NES, GC], f32r, tag="wq")
                wk_sb = wpool.tile([128, NES, GC], f32r, tag="wk")
                wv_sb = wpool.tile([128, NES, GC], f32r, tag="wv")
                for j in range(NES):
                    nc.sync.dma_start(wq_sb[:, j, :], wq_d[j * 128 : (j + 1) * 128, :])
                    nc.sync.dma_start(wk_sb[:, j, :], wk_d[j * 128 : (j + 1) * 128, :])
                    nc.sync.dma_start(wv_sb[:, j, :], wv_d[j * 128 : (j + 1) * 128, :])

                for st in range(NST):
                    xt = xtp.tile([128, NES, 512], f32r, tag="xt")
                    for ssl in range(4):
                        ss = st * 4 + ssl
                        xr = xrow.tile([128, E], f32, tag="xr")
                        nc.sync.dma_start(xr[:], x_d[ss * 128 : (ss + 1) * 128, :])
                        for jg in range(2):
                            xp = xps.tile([128, 4, 128], f32, tag="xp")
                            for jl in range(4):
                                j = jg * 4 + jl
                                nc.tensor.transpose(
                                    xp[:, jl, :],
                                    xr[:, j * 128 : (j + 1) * 128],
                                    ident[:],
                                )
                            nc.scalar.copy(
                                xt[:, jg * 4 : (jg + 1) * 4, ssl * 128 : (ssl + 1) * 128],
                                xp[:],
                            )
                    # q^T / k^T tiles: [128 cols, 512 s]
                    for m in range(NM):
                        pq = pps.tile([128, 512], f32, tag="pq")
                        for j in range(NES):
                            nc.tensor.matmul(
                                pq[:],
                                wq_sb[:, j, m * 128 : (m + 1) * 128],
                                xt[:, j, :],
                                start=(j == 0),
                                stop=(j == NES - 1),
                            )
                        nc.scalar.copy(qT[:, m, st * 512 : (st + 1) * 512], pq[:])
                        pk = pps.tile([128, 512], f32, tag="pk")
                        for j in range(NES):
                            nc.tensor.matmul(
                                pk[:],
                                wk_sb[:, j, m * 128 : (m + 1) * 128],
                                xt[:, j, :],
                                start=(j == 0),
                                stop=(j == NES - 1),
                            )
                        nc.scalar.copy(kT[:, m, st * 512 : (st + 1) * 512], pk[:])
                    # v tiles in natural [s, col] layout plus ones column
                    for ssl in range(4):
                        ss = st * 4 + ssl
                        pv = pps.tile([128, 512], f32, tag="pv")
                        for j in range(NES):
                            nc.tensor.matmul(
                                pv[:],
                                xt[:, j, ssl * 128 : (ssl + 1) * 128],
                                wv_sb[:, j, :],
                                start=(j == 0),
                                stop=(j == NES - 1),
                            )
                        nc.scalar.copy(
                            vsb[:, ss, :, 0:D],
                            pv[:].rearrange("p (h d) -> p h d", h=HL),
                        )
                        nc.vector.tensor_copy(
                            vsb[:, ss, :, D : D + 1], aux_sb[:, 0:HL, None]
                        )

            # ---- Phase 2: causal attention ----------------------------
            with (
                tc.tile_pool(name="sps", bufs=3, space="PSUM") as sps,
                tc.tile_pool(name="cps", bufs=2, space="PSUM") as cps,
                tc.tile_pool(name="bcs", bufs=2, space="PSUM") as bcs,
                tc.tile_pool(name="esb", bufs=4) as esb,
                tc.tile_pool(name="small", bufs=2) as small,
            ):
                for h in range(HL):
                    m, hr = h // 2, (h % 2) * D
                    for qi in range(NQT):
                        cP = cps.tile([D + 1, 512], f32, tag="cP")
                        nks = 4 * qi + 4
                        for ks in range(nks):
                            sP = sps.tile([128, 512], f32, tag="sP")
                            nc.tensor.matmul(
                                sP[:],
                                kT[hr : hr + D, m, ks * 128 : (ks + 1) * 128],
                                qT[hr : hr + D, m, qi * 512 : (qi + 1) * 512],
                                start=True,
                                stop=True,
                            )
                            eT = esb.tile([128, 512], f32r, tag="eT")
                            nc.scalar.activation(
                                eT[:],
                                sP[:],
                                AF.Exp,
                                bias=pad_sb[:, ks : ks + 1],
                                scale=SCALE,
                            )
                            if ks >= 4 * qi:  # diagonal tile: causal zero-fill
                                nc.gpsimd.affine_select(
                                    out=eT[:],
                                    in_=eT[:],
                                    compare_op=ALU.is_ge,
                                    fill=0.0,
                                    base=qi * 512 - ks * 128,
                                    pattern=[[1, 512]],
                                    channel_multiplier=-1,
                                )
                            nc.tensor.matmul(
                                cP[:],
                                vsb[:, ks, h, :],
                                eT[:],
                                start=(ks == 0),
                                stop=(ks == nks - 1),
                            )
                        rec = small.tile([1, 512], f32r, tag="rec")
                        with nc.allow_low_precision(reason="f32r keeps 19 bits"):
                            nc.vector.reciprocal(rec[:], cP[D : D + 1, :])
                        bc = bcs.tile([D, 512], f32, tag="bc")
                        nc.tensor.matmul(
                            bc[:], aux_sb[0:1, 0:D], rec[:], start=True, stop=True
                        )
                        bsb = small.tile([D, 512], f32, tag="bsb")
                        nc.scalar.copy(bsb[:], bc[:])
                        nc.vector.tensor_tensor(
                            out=ctxT[hr : hr + D, m, qi * 512 : (qi + 1) * 512],
                            in0=cP[0:D, :],
                            in1=bsb[:],
                            op=ALU.mult,
                        )

            # ---- Phase 3: partial out-projection ----------------------
            with (
                tc.tile_pool(name="wop", bufs=1) as wop,
                tc.tile_pool(name="ops", bufs=3, space="PSUM") as ops,
                tc.tile_pool(name="osb", bufs=3) as osb,
            ):
                wo_sb = wop.tile([128, NM, E], f32r, tag="wo")
                for m in range(NM):
                    nc.sync.dma_start(wo_sb[:, m, :], wo_d[m * 128 : (m + 1) * 128, :])
                for et in range(E // 128):
                    for st in range(NST):
                        oP = ops.tile([128, 512], f32, tag="oP")
                        for m in range(NM):
                            nc.tensor.matmul(
                                oP[:],
                                wo_sb[:, m, et * 128 : (et + 1) * 128],
                                ctxT[:, m, st * 512 : (st + 1) * 512],
                                start=(m == 0),
                                stop=(m == NM - 1),
                            )
                        ob = osb.tile([128, 512], f32, tag="ob")
                        nc.scalar.copy(ob[:], oP[:])
                        nc.sync.dma_start(
                            out_d[et * 128 : (et + 1) * 128, st * 512 : (st + 1) * 512],
                            ob[:],
                        )

    nc.finalize()
    return nc


LAST_RESULT = None
_LAST_IN_MAPS = None


def _in_maps(x, attention_mask, Wq, Wk, Wv, Wo):
    aux = np.ones((128, 64), dtype=np.float32)
    maps = []
    for c in range(8):
        b, g = c // 2, c % 2
        pad = np.where(np.asarray(attention_mask[b]) == 0, NEG, np.float32(0.0))
        pad = np.ascontiguousarray(
            pad.astype(np.float32).reshape(NKS, 128).T
        )  # [128, NKS]
        maps.append(
            {
                "x": np.ascontiguousarray(x[b]),
                "wq": np.ascontiguousarray(Wq[:, g * GC : (g + 1) * GC]),
                "wk": np.ascontiguousarray(Wk[:, g * GC : (g + 1) * GC]),
                "wv": np.ascontiguousarray(Wv[:, g * GC : (g + 1) * GC]),
                "wo": np.ascontiguousarray(Wo[g * GC : (g + 1) * GC, :]),
                "pad": pad,
                "aux": aux,
            }
        )
    return maps


def kernel(x, attention_mask, Wq, Wk, Wv, Wo, trace=False):
    global _CACHED_NC, LAST_RESULT, _LAST_IN_MAPS
    x = np.ascontiguousarray(np.asarray(x, dtype=np.float32))
    attention_mask = np.asarray(attention_mask)
    Wq = np.ascontiguousarray(np.asarray(Wq, dtype=np.float32))
    Wk = np.ascontiguousarray(np.asarray(Wk, dtype=np.float32))
    Wv = np.ascontiguousarray(np.asarray(Wv, dtype=np.float32))
    Wo = np.ascontiguousarray(np.asarray(Wo, dtype=np.float32))

    if _CACHED_NC is None:
        _CACHED_NC = _build_bass()
    nc = _CACHED_NC

    in_maps = _in_maps(x, attention_mask, Wq, Wk, Wv, Wo)
    _LAST_IN_MAPS = in_maps
    res = run_bass_kernel_spmd(nc, in_maps, core_ids=list(range(8)), trace=trace)
    LAST_RESULT = res
    outs = [r["outT"] for r in res.results]
    out = np.stack([(outs[2 * b] + outs[2 * b + 1]).T for b in range(B)])
    return out.astype(np.float32)


def bench(iters=10, nc=None, in_maps=None):
    """Time repeated executions of the compiled kernel via PJRT shard_map.

    Returns (times_ns list, outputs of last run as list of dicts). Inputs
    default to the nc/in_maps from the last kernel() call.
    """
    import time as _time

    import jax
    from jax.experimental.shard_map import shard_map
    from jax.sharding import Mesh, NamedSharding, PartitionSpec

    from concourse import bass2jax

    nc = nc or _CACHED_NC
    in_maps = in_maps or _LAST_IN_MAPS
    assert nc is not None and in_maps is not None, "call kernel() first"
    n_cores = len(in_maps)

    bass2jax.install_neuronx_cc_hook()
    partition_name = nc.partition_id_tensor.name if nc.partition_id_tensor else None
    in_names, out_names, out_avals, zero_outs = [], [], [], []
    for alloc in nc.m.functions[0].allocations:
        if not isinstance(alloc, mybir.MemoryLocationSet):
            continue
        name = alloc.memorylocations[0].name
        if alloc.kind == "ExternalInput":
            if name != partition_name:
                in_names.append(name)
        elif alloc.kind == "ExternalOutput":
            out_names.append(name)
            shape = tuple(alloc.tensor_shape)
            dtype = mybir.dt.np(alloc.dtype)
            out_avals.append(jax.core.ShapedArray(shape, dtype))
            zero_outs.append(np.zeros(shape, dtype))
    n_params = len(in_names)
    n_outs = len(out_avals)
    in_names = in_names + out_names
    if partition_name is not None:
        in_names.append(partition_name)
    donate = tuple(range(n_params, n_params + n_outs))

    def _body(*args):
        operands = list(args)
        if partition_name is not None:
            operands.append(bass2jax.partition_id_tensor())
        outs = bass2jax._bass_exec_p.bind(
            *operands,
            out_avals=tuple(out_avals),
            in_names=tuple(in_names),
            out_names=tuple(out_names),
            lowering_input_output_aliases=(),
            sim_require_finite=True,
            sim_require_nnan=True,
            nc=nc,
        )
        return tuple(outs)

    devices = jax.devices()[:n_cores]
    mesh = Mesh(np.asarray(devices), ("core",))
    in_specs = (PartitionSpec("core"),) * (n_params + n_outs)
    out_specs = (PartitionSpec("core"),) * len(out_names)
    sharded = jax.jit(
        shard_map(
            _body, mesh=mesh, in_specs=in_specs, out_specs=out_specs, check_rep=False
        ),
        donate_argnums=donate,
        keep_unused=True,
    )
    sh = NamedSharding(mesh, PartitionSpec("core"))
    concat_in = [
        jax.device_put(
            np.concatenate([np.asarray(in_maps[c][nm]) for c in range(n_cores)], 0), sh
        )
        for nm in in_names[:n_params]
    ]
    zsets = [
        [
            jax.device_put(np.zeros((n_cores * z.shape[0],) + z.shape[1:], z.dtype), sh)
            for z in zero_outs
        ]
        for _ in range(iters + 1)
    ]
    jax.block_until_ready(concat_in)
    jax.block_until_ready(zsets)

    outs = sharded(*concat_in, *zsets[0])  # warmup + compile
    jax.block_until_ready(outs)
    times = []
    for i in range(iters):
        t0 = _time.perf_counter()
        outs = sharded(*concat_in, *zsets[i + 1])
        jax.block_until_ready(outs)
        times.append((_time.perf_counter() - t0) * 1e9)
    results = []
    for c in range(n_cores):
        d = {}
        for nm, aval, arr in zip(out_names, out_avals, outs):
            rows = aval.shape[0]
            d[nm] = np.asarray(arr[c * rows : (c + 1) * rows])
        results.append(d)
    return times, results
